# revision 21
# baseline (speedup 1.0000x reference)
"""BiLSTM (2-layer, masked/ragged) Trainium2 kernel.

Sharding: 8 cores = 2 directions x 4 batch shards (BS=16 each). Backward
cores receive time-reversed inputs from the host, so the device program is
direction-agnostic SPMD. Layer-0 outputs are exchanged between fwd/bwd
partner cores with an 8-core AllGather of time-reversed copies.

Structure:
- Input projections (x@W_ih + b) write gate pre-activations straight into
  the scan's SBUF chunk tiles (no DRAM bounce); projection work for chunk
  c+1 is interleaved into the scan of chunk c, filling PE bubbles left by
  the serial cell chain. Projection finalize (bias+cast) alternates between
  ACT and DVE so neither queue head-of-line-blocks the scan chain.
- Scan: gates grouped g | i,f | o into three PSUM banks so each block's
  activation overlaps the next block's matmuls; the xw contribution is
  accumulated into PSUM by an identity-matmul (no DVE add); activations
  read PSUM directly. State masking rides on the Pool engine off the
  critical chain, which is: [64 whh MMs] -> sigmoid -> ig/fc/cn (DVE) ->
  tanh -> h_in (DVE) -> next step.
- xw and masks live in double-buffered per-chunk SBUF tiles; all stores
  are chunk-granular with 1KB-per-partition contiguous runs.
"""

import numpy as np
import ml_dtypes

import concourse.bass as bass
import concourse.bacc as bacc
import concourse.mybir as mybir
import concourse.tile as tile
from concourse import bass_utils

bf16 = ml_dtypes.bfloat16
f32 = mybir.dt.float32
bf = mybir.dt.bfloat16

T, B, D, H = 512, 64, 512, 512
NCORES = 8
BS = B // 4          # 16 batch per core
G = 4 * H            # 2048 gates
GT = G // 128        # 16 gate tiles
KH = H // 128        # 4 contraction chunks for H
TC = 32              # timesteps per chunk
FD = TC * BS         # 512, proj matmul free dim

Tanh = mybir.ActivationFunctionType.Tanh
Sigmoid = mybir.ActivationFunctionType.Sigmoid
Identity = mybir.ActivationFunctionType.Identity

_compiled = {}


def _build(t_steps=T):
    assert t_steps % TC == 0
    NC = t_steps // TC
    nc = bacc.Bacc("TRN2", target_bir_lowering=False, debug=False,
                   num_devices=NCORES)

    # ---- per-core external inputs ----
    xT = nc.dram_tensor("xT", (D, t_steps, BS), bf, kind="ExternalInput")
    maskd = nc.dram_tensor("maskd", (NC, (TC + 1) * KH * BS), bf,
                           kind="ExternalInput")
    whh0T = nc.dram_tensor("whh0T", (KH, 128, G), bf, kind="ExternalInput")
    wih0T = nc.dram_tensor("wih0T", (KH, 128, G), bf, kind="ExternalInput")
    whh1T = nc.dram_tensor("whh1T", (KH, 128, G), bf, kind="ExternalInput")
    wih1oT = nc.dram_tensor("wih1oT", (KH, 128, G), bf, kind="ExternalInput")
    wih1pT = nc.dram_tensor("wih1pT", (KH, 128, G), bf, kind="ExternalInput")
    ident = nc.dram_tensor("ident", (128, 128), bf, kind="ExternalInput")
    b0c = nc.dram_tensor("b0c", (GT, 128), f32, kind="ExternalInput")
    b1c = nc.dram_tensor("b1c", (GT, 128), f32, kind="ExternalInput")
    y1 = nc.dram_tensor("y1", (KH, NC, 128, TC, BS), bf,
                        kind="ExternalOutput")

    with tile.TileContext(nc) as tc:
        with (
            tc.tile_pool(name="wpool", bufs=1) as wpool,
            tc.tile_pool(name="xwpool", bufs=2) as xwpool,
            tc.tile_pool(name="mpool", bufs=2) as mpool,
            tc.tile_pool(name="rhspool", bufs=2) as rhspool,
            tc.tile_pool(name="ypool", bufs=2) as ypool,
            tc.tile_pool(name="spool", bufs=3) as spool,
            tc.tile_pool(name="state", bufs=1) as state,
            tc.tile_pool(name="psS", bufs=1, space="PSUM") as psS,
            tc.tile_pool(name="psA", bufs=2, space="PSUM") as psA,
            tc.tile_pool(name="dram", bufs=1, space="DRAM") as dram,
        ):
            # ---- internal DRAM ----
            y0 = dram.tile([KH, NC, 128, TC, BS], bf)
            y0x = dram.tile([KH * NC, 128, TC, BS], bf)
            ag = dram.tile([NCORES * KH * NC, 128, TC, BS], bf,
                           addr_space="Shared")

            # ---- resident weights ----
            def load_w(name, src, dt_=bf):
                t = wpool.tile([128, KH * G], dt_, tag=name, name=name)
                for k in range(KH):
                    nc.sync.dma_start(t[:, k * G:(k + 1) * G], src.ap()[k])
                return t

            whh0_sb = load_w("whh0", whh0T)
            wih0_sb = load_w("wih0", wih0T)
            whh1_sb = load_w("whh1", whh1T)
            wih1o_sb = load_w("wih1o", wih1oT)
            wih1p_sb = load_w("wih1p", wih1pT)
            ident_sb = wpool.tile([128, 128], bf, tag="ident")
            nc.sync.dma_start(ident_sb[:], ident.ap())
            bias_sb = wpool.tile([128, 2 * GT], f32, tag="bias")
            nc.sync.dma_start(bias_sb[:, 0:GT], b0c.ap().transpose([1, 0]))
            nc.sync.dma_start(bias_sb[:, GT:2 * GT],
                              b1c.ap().transpose([1, 0]))

            partner_base = nc.snap(
                ((nc.partition_id() + 4) % NCORES) * (KH * NC))

            # ---------- projection codegen ----------
            def stage_rhs_l0(c):
                """Stage x chunk c into SBUF: [128, KH, TC, BS]."""
                rhs = rhspool.tile([128, KH, TC, BS], bf, tag="rhs0")
                t0 = c * TC
                for k in range(KH):
                    nc.sync.dma_start(
                        rhs[:, k],
                        xT.ap()[k * 128:(k + 1) * 128, t0:t0 + TC, :])
                return rhs

            def stage_rhs_l1(c):
                orhs = rhspool.tile([128, KH, TC, BS], bf, tag="rhs1o")
                nc.sync.dma_start(orhs[:],
                                  y0[:, c].transpose([1, 0, 2, 3]))
                prhs = rhspool.tile([128, KH, TC, BS], bf, tag="rhs1p")
                for k in range(KH):
                    nc.sync.dma_start(
                        prhs[:, k:k + 1],
                        ag[bass.ds(partner_base + k * NC + c, 1)]
                        .transpose([1, 0, 2, 3]))
                return orhs, prhs

            # xw storage block for natural gate tile g=(q*KH+j):
            # i->0..3, f->4..7, o->8..11, g(cand)->12..15
            def _sb_block(g):
                q, j = g // KH, g % KH
                return {0: 0, 1: KH, 2: 3 * KH, 3: 2 * KH}[q] + j

            def proj_tile_parts(g, w_rhs, xw_dst, bias_col):
                """One gate tile's projection as two emission parts, so the
                interleaved PE/ACT/DVE bursts stay small: part A = first half
                of the accumulation, part B = rest + two half-width
                bias-casts (alternating ACT/DVE across tiles)."""
                ps = [None]
                pairs = [(w_sb, rhs, k) for w_sb, rhs in w_rhs
                         for k in range(KH)]
                nk = len(pairs)
                half = nk // 2

                def emit(lo, hi):
                    for ji in range(lo, hi):
                        w_sb, rhs, k = pairs[ji]
                        nc.tensor.matmul(
                            ps[0][:],
                            w_sb[:, k * G + g * 128: k * G + (g + 1) * 128],
                            rhs[:, k],
                            start=(ji == 0),
                            stop=(ji == nk - 1),
                        )

                def part_a():
                    ps[0] = psA.tile([128, TC * BS], f32, tag="psA",
                                     name="psA")
                    emit(0, half)

                def part_b():
                    emit(half, nk)
                    HF = TC * BS // 2
                    blk = _sb_block(g)
                    bias = bias_sb[:, bias_col + g: bias_col + g + 1]
                    for piece in range(2):
                        sl = slice(piece * HF, (piece + 1) * HF)
                        if (g + piece) % 2 == 0:
                            nc.scalar.activation(
                                xw_dst[:, blk, sl], ps[0][:, sl], Identity,
                                bias=bias)
                        else:
                            nc.vector.tensor_scalar_add(
                                xw_dst[:, blk, sl], ps[0][:, sl], bias)

                return part_a, part_b

            def proj_tile(g, w_rhs, xw_dst, bias_col):
                a, b = proj_tile_parts(g, w_rhs, xw_dst, bias_col)
                a()
                b()

            def new_xw_tile():
                return xwpool.tile([128, GT, TC * BS], bf, tag="xw", name="xw")

            def new_mask_tile(c):
                m = mpool.tile([128, (TC + 1) * KH * BS], bf, tag="m", name="m")
                nc.sync.dma_start(
                    m[:],
                    maskd.ap()[c:c + 1, :].broadcast_to(
                        [128, (TC + 1) * KH * BS]))
                return m

            # ---------- scan codegen ----------
            def scan(whh_sb, xw_tiles, m_tiles, layer):
                """Single-chain scan: 3 PSUM banks (g | i+f | o) so each
                block's activation overlaps the next block's matmuls; xw adds
                folded into the PE via identity-matmuls; mask muls on Pool."""
                h_in = state.tile([128, KH, BS], bf, tag=f"hin{layer}",
                                  name="hin")
                cst = state.tile([128, KH, BS], f32, tag=f"cst{layer}",
                                 name="cst")
                nc.gpsimd.memset(h_in[:], 0.0)
                nc.gpsimd.memset(cst[:], 0.0)
                ps_g = ps_if = ps_o = None

                for c in range(NC):
                    xw = xw_tiles[c]
                    m_sb = m_tiles[c]
                    ych = ypool.tile([128, KH, TC, BS], bf, tag="ych",
                                     name="ych")
                    if layer == 0:
                        ychr = ypool.tile([128, KH, TC, BS], bf, tag="ychr",
                                          name="ychr")
                    if c + 1 < NC and m_tiles[c + 1] is None:
                        m_tiles[c + 1] = new_mask_tile(c + 1)
                    hooks = _hooks(layer, c) if c + 1 < NC else {}

                    for sidx in range(TC):
                        t = c * TC + sidx
                        last = (t == t_steps - 1)
                        ps_g = psS.tile([128, KH, BS], f32, tag="psg",
                                        name="psg", bufs=2)
                        ps_if = psS.tile([128, 2 * KH, BS], f32, tag="psif",
                                         name="psif", bufs=2)
                        ps_o = psS.tile([128, KH, BS], f32, tag="pso",
                                        name="pso", bufs=2)
                        mb = KH * BS
                        m_cur = m_sb[:, sidx * mb:(sidx + 1) * mb].rearrange(
                            "p (j b) -> p j b", j=KH)
                        m_next = m_sb[:, (sidx + 1) * mb:
                                      (sidx + 2) * mb].rearrange(
                            "p (j b) -> p j b", j=KH)

                        def block_mms(pst, tiles, xw_lo, xw_hi):
                            for ji, gt in enumerate(tiles):
                                for k in range(KH):
                                    nc.tensor.matmul(
                                        pst[:, ji, :],
                                        whh_sb[:, k * G + gt * 128:
                                               k * G + (gt + 1) * 128],
                                        h_in[:, k, :],
                                        start=(ji == 0 and k == 0),
                                        stop=False,
                                        skip_group_check=True)
                            nc.tensor.matmul(
                                pst[:], ident_sb[:],
                                xw[:, xw_lo:xw_hi,
                                   sidx * BS:(sidx + 1) * BS],
                                start=False, stop=True,
                                skip_group_check=True)

                        block_mms(ps_g, list(range(2 * KH, 3 * KH)), 12, 16)
                        tng = spool.tile([128, KH, BS], f32, tag="tng",
                                         name="tng")
                        nc.scalar.activation(tng[:], ps_g[:], Tanh)

                        block_mms(ps_if, list(range(0, 2 * KH)), 0, 8)
                        sif = spool.tile([128, 2 * KH, BS], f32, tag="sif",
                                         name="sif")
                        nc.scalar.activation(sif[:], ps_if[:], Sigmoid)

                        block_mms(ps_o, list(range(3 * KH, 4 * KH)), 8, 12)
                        sfo = spool.tile([128, KH, BS], f32, tag="sfo",
                                         name="sfo")
                        nc.scalar.activation(sfo[:], ps_o[:], Sigmoid)

                        ig = spool.tile([128, KH, BS], f32, tag="ig",
                                        name="ig")
                        nc.vector.tensor_mul(ig[:], sif[:, 0:KH], tng[:])
                        fc = spool.tile([128, KH, BS], f32, tag="fc",
                                        name="fc")
                        nc.vector.tensor_mul(fc[:], sif[:, KH:2 * KH],
                                             cst[:])
                        cn = spool.tile([128, KH, BS], f32, tag="cn",
                                        name="cn")
                        nc.vector.tensor_add(cn[:], ig[:], fc[:])
                        tc2 = spool.tile([128, KH, BS], f32, tag="tc2",
                                         name="tc2")
                        nc.scalar.activation(tc2[:], cn[:], Tanh)
                        if not last:
                            nc.gpsimd.tensor_mul(cst[:], cn[:], m_next)
                            sfom = spool.tile([128, KH, BS], f32,
                                              tag="sfom", name="sfom")
                            nc.gpsimd.tensor_mul(sfom[:], sfo[:], m_next)
                            nc.vector.tensor_mul(h_in[:], sfom[:], tc2[:])

                        ysl = ych[:, :, sidx, :]
                        if layer == 0:
                            t2y = spool.tile([128, KH, BS], f32, tag="t2y",
                                             name="t2y")
                            nc.gpsimd.tensor_mul(t2y[:], tc2[:], m_cur)
                            nc.gpsimd.tensor_mul(ysl, sfo[:], t2y[:])
                            nc.gpsimd.tensor_copy(
                                ychr[:, :, TC - 1 - sidx, :], ysl)
                        else:
                            nc.gpsimd.tensor_mul(ysl, sfo[:], tc2[:])

                        hook = hooks.get(sidx)
                        if hook:
                            hook()

                    if layer == 0:
                        nc.sync.dma_start(
                            y0[:, c].transpose([1, 0, 2, 3]), ych[:])
                        cr = NC - 1 - c
                        for k in range(KH):
                            nc.sync.dma_start(y0x[k * NC + cr], ychr[:, k])
                    else:
                        nc.sync.dma_start(
                            y1.ap()[:, c].transpose([1, 0, 2, 3]), ych[:])

            # hooks: spread chunk c+1's projection over chunk c's steps,
            # two part-emissions per gate tile so each slot's burst is small
            def _hooks(layer, c):
                cn_ = c + 1
                hooks = {}
                st = {}
                if layer == 0:
                    def stage():
                        st["r"] = [(wih0_sb, stage_rhs_l0(cn_))]
                        xw_tiles0[cn_] = new_xw_tile()
                        st["xw"] = xw_tiles0[cn_]
                    bias_col = 0
                else:
                    def stage():
                        orhs, prhs = stage_rhs_l1(cn_)
                        st["r"] = [(wih1o_sb, orhs), (wih1p_sb, prhs)]
                        xw_tiles1[cn_] = new_xw_tile()
                        st["xw"] = xw_tiles1[cn_]
                    bias_col = GT
                hooks[0] = stage

                parts = []

                def mk(gidx):
                    holder = {}

                    def part_a():
                        a, b = proj_tile_parts(gidx, st["r"], st["xw"],
                                               bias_col)
                        holder["b"] = b
                        a()

                    def part_b():
                        holder["b"]()
                    return part_a, part_b

                for g in range(GT):
                    a, b = mk(g)
                    parts.append(a)
                    parts.append(b)
                # 32 parts over slots 2..TC-1 (30 slots): double-up the
                # first two slots
                slots = list(range(2, TC))
                while len(parts) > len(slots):
                    f1, f2 = parts[0], parts[1]
                    parts = [lambda f1=f1, f2=f2: (f1(), f2())] + parts[2:]
                for sl, fn in zip(slots, parts):
                    hooks[sl] = fn
                return hooks

            # ---------- phase sequence ----------
            # prologue: proj chunk 0 of layer 0
            xw_tiles0 = [None] * NC
            xw_tiles1 = [None] * NC
            m_tiles = [None] * NC
            m_tiles[0] = new_mask_tile(0)
            rhs0 = stage_rhs_l0(0)
            xw_tiles0[0] = new_xw_tile()
            for g in range(GT):
                proj_tile(g, [(wih0_sb, rhs0)], xw_tiles0[0], bias_col=0)

            scan(whh0_sb, xw_tiles0, m_tiles, layer=0)

            # exchange
            nc.gpsimd.collective_compute(
                "AllGather", mybir.AluOpType.bypass,
                ins=[y0x.opt()], outs=[ag.opt()],
                replica_groups=[list(range(NCORES))],
            )

            # prologue: proj chunk 0 of layer 1
            m_tiles2 = [None] * NC
            m_tiles2[0] = new_mask_tile(0)
            orhs0, prhs0 = stage_rhs_l1(0)
            xw_tiles1[0] = new_xw_tile()
            for g in range(GT):
                proj_tile(g, [(wih1o_sb, orhs0), (wih1p_sb, prhs0)],
                          xw_tiles1[0], bias_col=GT)

            scan(whh1_sb, xw_tiles1, m_tiles2, layer=1)

    nc.compile()
    return nc


def _prep_inputs(x, lengths, weights, t_steps=T):
    """Build the 8 per-core input maps."""
    NC = t_steps // TC
    x = np.asarray(x, np.float32)
    lengths = np.asarray(lengths)
    active = (np.arange(T)[:, None] < lengths[None, :]).astype(np.float32)

    per_dir = {}
    for d, pre in ((0, "f"), (1, "b")):
        xs = x if d == 0 else x[::-1]
        am = active if d == 0 else active[::-1]
        xs = xs[:t_steps]
        am = am[:t_steps]
        xTd = np.ascontiguousarray(
            xs.transpose(2, 0, 1)).astype(bf16)  # [D, t, B]
        # mask rows: chunk c covers steps [c*TC, c*TC+TC] inclusive
        amp = np.vstack([am, np.ones((1, B), np.float32)])
        W_ih0 = np.asarray(weights[f"{pre}W_ih0"], np.float32)
        W_hh0 = np.asarray(weights[f"{pre}W_hh0"], np.float32)
        W_ih1 = np.asarray(weights[f"{pre}W_ih1"], np.float32)
        W_hh1 = np.asarray(weights[f"{pre}W_hh1"], np.float32)
        own = W_ih1[:, :H] if d == 0 else W_ih1[:, H:]
        par = W_ih1[:, H:] if d == 0 else W_ih1[:, :H]
        per_dir[d] = dict(
            xT=xTd, amp=amp,
            whh0T=np.ascontiguousarray(
                W_hh0.T.reshape(KH, 128, G)).astype(bf16),
            wih0T=np.ascontiguousarray(
                W_ih0.T.reshape(KH, 128, G)).astype(bf16),
            whh1T=np.ascontiguousarray(
                W_hh1.T.reshape(KH, 128, G)).astype(bf16),
            wih1oT=np.ascontiguousarray(
                own.T.reshape(KH, 128, G)).astype(bf16),
            wih1pT=np.ascontiguousarray(
                par.T.reshape(KH, 128, G)).astype(bf16),
            b0c=np.ascontiguousarray(
                np.asarray(weights[f"{pre}b0"],
                           np.float32).reshape(GT, 128)),
            b1c=np.ascontiguousarray(
                np.asarray(weights[f"{pre}b1"],
                           np.float32).reshape(GT, 128)),
        )

    in_maps = []
    for core in range(NCORES):
        d, s = core // 4, core % 4
        bsl = slice(s * BS, (s + 1) * BS)
        pd = per_dir[d]
        ams = pd["amp"][:, bsl]  # [t_steps+1, BS]
        maskrows = np.empty((NC, (TC + 1) * KH * BS), np.float32)
        for c in range(NC):
            blk = ams[c * TC:c * TC + TC + 1]            # [TC+1, BS]
            maskrows[c] = np.tile(blk, (1, KH)).reshape(-1)
        in_maps.append({
            "ident": np.eye(128, dtype=bf16),
            "xT": np.ascontiguousarray(pd["xT"][:, :, bsl]),
            "maskd": maskrows.astype(bf16),
            "whh0T": pd["whh0T"],
            "wih0T": pd["wih0T"],
            "whh1T": pd["whh1T"],
            "wih1oT": pd["wih1oT"],
            "wih1pT": pd["wih1pT"],
            "b0c": pd["b0c"],
            "b1c": pd["b1c"],
        })
    return in_maps


def _assemble(results, lengths, t_steps=T):
    NC = t_steps // TC
    lengths = np.asarray(lengths)
    active = (np.arange(t_steps)[:, None] < lengths[None, :])
    out = np.zeros((t_steps, B, 2 * H), np.float32)
    for core in range(NCORES):
        d, s = core // 4, core % 4
        arr = np.asarray(results[core]["y1"], dtype=bf16).astype(np.float32)
        # [KH, NC, 128, TC, BS] -> [t, b, j*128+p]
        blk = arr.transpose(1, 3, 4, 0, 2).reshape(t_steps, BS, H)
        if d == 1:
            blk = blk[::-1]
        out[:, s * BS:(s + 1) * BS, d * H:(d + 1) * H] = blk
    out *= active[:, :, None]
    return out


def kernel(x, lengths, fW_ih0, fW_hh0, fb0, bW_ih0, bW_hh0, bb0,
           fW_ih1, fW_hh1, fb1, bW_ih1, bW_hh1, bb1, _t_steps=T,
           _want_trace=False):
    weights = dict(fW_ih0=fW_ih0, fW_hh0=fW_hh0, fb0=fb0,
                   bW_ih0=bW_ih0, bW_hh0=bW_hh0, bb0=bb0,
                   fW_ih1=fW_ih1, fW_hh1=fW_hh1, fb1=fb1,
                   bW_ih1=bW_ih1, bW_hh1=bW_hh1, bb1=bb1)
    key = _t_steps
    if key not in _compiled:
        _compiled[key] = _build(_t_steps)
    nc = _compiled[key]
    in_maps = _prep_inputs(x, lengths, weights, _t_steps)
    res = bass_utils.run_bass_kernel_spmd(
        nc, in_maps, core_ids=list(range(NCORES)), trace=_want_trace)
    out = _assemble(res.results, lengths, _t_steps)
    if _want_trace:
        kernel.last_results = res
    return out


# revision 22
# speedup vs baseline: 1.0368x; 1.0368x over previous
"""BiLSTM (2-layer, masked/ragged) Trainium2 kernel.

Sharding: 8 cores = 2 directions x 4 batch shards (BS=16 each). Backward
cores receive time-reversed inputs from the host, so the device program is
direction-agnostic SPMD. Layer-0 outputs are exchanged between fwd/bwd
partner cores with an 8-core AllGather of time-reversed copies.

Structure:
- Input projections (x@W_ih + b) write gate pre-activations straight into
  the scan's SBUF chunk tiles (no DRAM bounce); projection work for chunk
  c+1 is interleaved into the scan of chunk c, filling PE bubbles left by
  the serial cell chain. Projection finalize (bias+cast) alternates between
  ACT and DVE so neither queue head-of-line-blocks the scan chain.
- Scan: gates grouped g | i,f | o into three PSUM banks so each block's
  activation overlaps the next block's matmuls; the xw contribution is
  accumulated into PSUM by an identity-matmul (no DVE add); activations
  read PSUM directly. State masking rides on the Pool engine off the
  critical chain, which is: [64 whh MMs] -> sigmoid -> ig/fc/cn (DVE) ->
  tanh -> h_in (DVE) -> next step.
- xw and masks live in double-buffered per-chunk SBUF tiles; all stores
  are chunk-granular with 1KB-per-partition contiguous runs.
"""

import numpy as np
import ml_dtypes

import concourse.bass as bass
import concourse.bacc as bacc
import concourse.mybir as mybir
import concourse.tile as tile
from concourse import bass_utils

bf16 = ml_dtypes.bfloat16
f32 = mybir.dt.float32
bf = mybir.dt.bfloat16

T, B, D, H = 512, 64, 512, 512
NCORES = 8
BS = B // 4          # 16 batch per core
G = 4 * H            # 2048 gates
GT = G // 128        # 16 gate tiles
KH = H // 128        # 4 contraction chunks for H
TC = 32              # timesteps per chunk
FD = TC * BS         # 512, proj matmul free dim

Tanh = mybir.ActivationFunctionType.Tanh
Sigmoid = mybir.ActivationFunctionType.Sigmoid
Identity = mybir.ActivationFunctionType.Identity

_compiled = {}


def _build(t_steps=T):
    assert t_steps % TC == 0
    NC = t_steps // TC
    nc = bacc.Bacc("TRN2", target_bir_lowering=False, debug=False,
                   num_devices=NCORES)

    # ---- per-core external inputs ----
    xT = nc.dram_tensor("xT", (D, t_steps, BS), bf, kind="ExternalInput")
    maskd = nc.dram_tensor("maskd", (NC, (TC + 1) * KH * BS), bf,
                           kind="ExternalInput")
    whh0T = nc.dram_tensor("whh0T", (KH, 128, G), bf, kind="ExternalInput")
    wih0T = nc.dram_tensor("wih0T", (KH, 128, G), bf, kind="ExternalInput")
    whh1T = nc.dram_tensor("whh1T", (KH, 128, G), bf, kind="ExternalInput")
    wih1oT = nc.dram_tensor("wih1oT", (KH, 128, G), bf, kind="ExternalInput")
    wih1pT = nc.dram_tensor("wih1pT", (KH, 128, G), bf, kind="ExternalInput")
    ident = nc.dram_tensor("ident", (128, 128), bf, kind="ExternalInput")
    b0c = nc.dram_tensor("b0c", (GT, 128), f32, kind="ExternalInput")
    b1c = nc.dram_tensor("b1c", (GT, 128), f32, kind="ExternalInput")
    y1 = nc.dram_tensor("y1", (KH, NC, 128, TC, BS), bf,
                        kind="ExternalOutput")

    with tile.TileContext(nc) as tc:
        with (
            tc.tile_pool(name="wpool", bufs=1) as wpool,
            tc.tile_pool(name="xwpool", bufs=2) as xwpool,
            tc.tile_pool(name="mpool", bufs=2) as mpool,
            tc.tile_pool(name="rhspool", bufs=2) as rhspool,
            tc.tile_pool(name="ypool", bufs=2) as ypool,
            tc.tile_pool(name="spool", bufs=3) as spool,
            tc.tile_pool(name="state", bufs=1) as state,
            tc.tile_pool(name="psS", bufs=1, space="PSUM") as psS,
            tc.tile_pool(name="psA", bufs=2, space="PSUM") as psA,
            tc.tile_pool(name="dram", bufs=1, space="DRAM") as dram,
        ):
            # ---- internal DRAM ----
            y0 = dram.tile([KH, NC, 128, TC, BS], bf)
            y0x = dram.tile([KH * NC, 128, TC, BS], bf)
            ag = dram.tile([NCORES * KH * NC, 128, TC, BS], bf,
                           addr_space="Shared")

            # ---- resident weights ----
            def load_w(name, src, dt_=bf):
                t = wpool.tile([128, KH * G], dt_, tag=name, name=name)
                for k in range(KH):
                    nc.sync.dma_start(t[:, k * G:(k + 1) * G], src.ap()[k])
                return t

            whh0_sb = load_w("whh0", whh0T)
            wih0_sb = load_w("wih0", wih0T)
            whh1_sb = load_w("whh1", whh1T)
            wih1o_sb = load_w("wih1o", wih1oT)
            wih1p_sb = load_w("wih1p", wih1pT)
            ident_sb = wpool.tile([128, 128], bf, tag="ident")
            nc.sync.dma_start(ident_sb[:], ident.ap())
            bias_sb = wpool.tile([128, 2 * GT], f32, tag="bias")
            nc.sync.dma_start(bias_sb[:, 0:GT], b0c.ap().transpose([1, 0]))
            nc.sync.dma_start(bias_sb[:, GT:2 * GT],
                              b1c.ap().transpose([1, 0]))

            partner_base = nc.snap(
                ((nc.partition_id() + 4) % NCORES) * (KH * NC))

            # ---------- projection codegen ----------
            def stage_rhs_l0(c):
                """Stage x chunk c into SBUF: [128, KH, TC, BS]."""
                rhs = rhspool.tile([128, KH, TC, BS], bf, tag="rhs0")
                t0 = c * TC
                for k in range(KH):
                    nc.sync.dma_start(
                        rhs[:, k],
                        xT.ap()[k * 128:(k + 1) * 128, t0:t0 + TC, :])
                return rhs

            def stage_rhs_l1(c):
                orhs = rhspool.tile([128, KH, TC, BS], bf, tag="rhs1o")
                nc.sync.dma_start(orhs[:],
                                  y0[:, c].transpose([1, 0, 2, 3]))
                prhs = rhspool.tile([128, KH, TC, BS], bf, tag="rhs1p")
                for k in range(KH):
                    nc.sync.dma_start(
                        prhs[:, k:k + 1],
                        ag[bass.ds(partner_base + k * NC + c, 1)]
                        .transpose([1, 0, 2, 3]))
                return orhs, prhs

            # xw storage block for natural gate tile g=(q*KH+j):
            # i->0..3, f->4..7, o->8..11, g(cand)->12..15
            def _sb_block(g):
                q, j = g // KH, g % KH
                return {0: 0, 1: KH, 2: 3 * KH, 3: 2 * KH}[q] + j

            def proj_tile_parts(g, w_rhs, xw_dst, bias_col):
                """One gate tile's projection as two emission parts, so the
                interleaved PE/ACT/DVE bursts stay small: part A = first half
                of the accumulation, part B = rest + two half-width
                bias-casts (alternating ACT/DVE across tiles)."""
                ps = [None]
                pairs = [(w_sb, rhs, k) for w_sb, rhs in w_rhs
                         for k in range(KH)]
                nk = len(pairs)
                half = nk // 2

                def emit(lo, hi):
                    for ji in range(lo, hi):
                        w_sb, rhs, k = pairs[ji]
                        nc.tensor.matmul(
                            ps[0][:],
                            w_sb[:, k * G + g * 128: k * G + (g + 1) * 128],
                            rhs[:, k],
                            start=(ji == 0),
                            stop=(ji == nk - 1),
                        )

                def part_a():
                    ps[0] = psA.tile([128, TC * BS], f32, tag="psA",
                                     name="psA")
                    emit(0, half)

                def part_b():
                    emit(half, nk)
                    HF = TC * BS // 2
                    blk = _sb_block(g)
                    bias = bias_sb[:, bias_col + g: bias_col + g + 1]
                    for piece in range(2):
                        sl = slice(piece * HF, (piece + 1) * HF)
                        if (g + piece) % 2 == 0:
                            nc.scalar.activation(
                                xw_dst[:, blk, sl], ps[0][:, sl], Identity,
                                bias=bias)
                        else:
                            nc.vector.tensor_scalar_add(
                                xw_dst[:, blk, sl], ps[0][:, sl], bias)

                return part_a, part_b

            def proj_tile(g, w_rhs, xw_dst, bias_col):
                a, b = proj_tile_parts(g, w_rhs, xw_dst, bias_col)
                a()
                b()

            def new_xw_tile():
                return xwpool.tile([128, GT, TC * BS], bf, tag="xw", name="xw")

            def new_mask_tile(c):
                m = mpool.tile([128, (TC + 1) * KH * BS], bf, tag="m", name="m")
                nc.sync.dma_start(
                    m[:],
                    maskd.ap()[c:c + 1, :].broadcast_to(
                        [128, (TC + 1) * KH * BS]))
                return m

            # ---------- scan codegen ----------
            def scan(whh_sb, xw_tiles, m_tiles, layer):
                """Single-chain scan: 3 PSUM banks (g | i+f | o) so each
                block's activation overlaps the next block's matmuls; xw adds
                folded into the PE via identity-matmuls; mask muls on Pool."""
                h_in = state.tile([128, KH, BS], bf, tag=f"hin{layer}",
                                  name="hin")
                cst = state.tile([128, KH, BS], f32, tag=f"cst{layer}",
                                 name="cst")
                nc.gpsimd.memset(h_in[:], 0.0)
                nc.gpsimd.memset(cst[:], 0.0)
                ps_g = psS.tile([128, KH, BS], f32, tag="psg", name="psg")
                ps_if = psS.tile([128, 2 * KH, BS], f32, tag="psif",
                                 name="psif")
                ps_o = psS.tile([128, KH, BS], f32, tag="pso", name="pso")

                for c in range(NC):
                    xw = xw_tiles[c]
                    m_sb = m_tiles[c]
                    ych = ypool.tile([128, KH, TC, BS], bf, tag="ych",
                                     name="ych")
                    if layer == 0:
                        ychr = ypool.tile([128, KH, TC, BS], bf, tag="ychr",
                                          name="ychr")
                    if c + 1 < NC and m_tiles[c + 1] is None:
                        m_tiles[c + 1] = new_mask_tile(c + 1)
                    hooks = _hooks(layer, c) if c + 1 < NC else {}

                    for sidx in range(TC):
                        t = c * TC + sidx
                        last = (t == t_steps - 1)
                        mb = KH * BS
                        m_cur = m_sb[:, sidx * mb:(sidx + 1) * mb].rearrange(
                            "p (j b) -> p j b", j=KH)
                        m_next = m_sb[:, (sidx + 1) * mb:
                                      (sidx + 2) * mb].rearrange(
                            "p (j b) -> p j b", j=KH)

                        def block_mms(pst, tiles, xw_lo, xw_hi):
                            for ji, gt in enumerate(tiles):
                                for k in range(KH):
                                    nc.tensor.matmul(
                                        pst[:, ji, :],
                                        whh_sb[:, k * G + gt * 128:
                                               k * G + (gt + 1) * 128],
                                        h_in[:, k, :],
                                        start=(ji == 0 and k == 0),
                                        stop=False,
                                        skip_group_check=True)
                            nc.tensor.matmul(
                                pst[:], ident_sb[:],
                                xw[:, xw_lo:xw_hi,
                                   sidx * BS:(sidx + 1) * BS],
                                start=False, stop=True,
                                skip_group_check=True)

                        block_mms(ps_g, list(range(2 * KH, 3 * KH)), 12, 16)
                        tng = spool.tile([128, KH, BS], f32, tag="tng",
                                         name="tng")
                        nc.scalar.activation(tng[:], ps_g[:], Tanh)

                        block_mms(ps_if, list(range(0, 2 * KH)), 0, 8)
                        sif = spool.tile([128, 2 * KH, BS], f32, tag="sif",
                                         name="sif")
                        nc.scalar.activation(sif[:], ps_if[:], Sigmoid)

                        block_mms(ps_o, list(range(3 * KH, 4 * KH)), 8, 12)
                        sfo = spool.tile([128, KH, BS], f32, tag="sfo",
                                         name="sfo")
                        nc.scalar.activation(sfo[:], ps_o[:], Sigmoid)

                        ig = spool.tile([128, KH, BS], f32, tag="ig",
                                        name="ig")
                        nc.vector.tensor_mul(ig[:], sif[:, 0:KH], tng[:])
                        fc = spool.tile([128, KH, BS], f32, tag="fc",
                                        name="fc")
                        nc.vector.tensor_mul(fc[:], sif[:, KH:2 * KH],
                                             cst[:])
                        cn = spool.tile([128, KH, BS], f32, tag="cn",
                                        name="cn")
                        nc.vector.tensor_add(cn[:], ig[:], fc[:])
                        tc2 = spool.tile([128, KH, BS], f32, tag="tc2",
                                         name="tc2")
                        nc.scalar.activation(tc2[:], cn[:], Tanh)
                        if not last:
                            nc.gpsimd.tensor_mul(cst[:], cn[:], m_next)
                            sfom = spool.tile([128, KH, BS], f32,
                                              tag="sfom", name="sfom")
                            nc.gpsimd.tensor_mul(sfom[:], sfo[:], m_next)
                            nc.vector.tensor_mul(h_in[:], sfom[:], tc2[:])

                        ysl = ych[:, :, sidx, :]
                        if layer == 0:
                            t2y = spool.tile([128, KH, BS], f32, tag="t2y",
                                             name="t2y")
                            nc.gpsimd.tensor_mul(t2y[:], tc2[:], m_cur)
                            nc.gpsimd.tensor_mul(ysl, sfo[:], t2y[:])
                            nc.gpsimd.tensor_copy(
                                ychr[:, :, TC - 1 - sidx, :], ysl)
                        else:
                            nc.gpsimd.tensor_mul(ysl, sfo[:], tc2[:])

                        hook = hooks.get(sidx)
                        if hook:
                            hook()

                    if layer == 0:
                        nc.sync.dma_start(
                            y0[:, c].transpose([1, 0, 2, 3]), ych[:])
                        cr = NC - 1 - c
                        for k in range(KH):
                            nc.sync.dma_start(y0x[k * NC + cr], ychr[:, k])
                    else:
                        nc.sync.dma_start(
                            y1.ap()[:, c].transpose([1, 0, 2, 3]), ych[:])

            # hooks: spread chunk c+1's projection over chunk c's steps,
            # two part-emissions per gate tile so each slot's burst is small
            def _hooks(layer, c):
                cn_ = c + 1
                hooks = {}
                st = {}
                if layer == 0:
                    def stage():
                        st["r"] = [(wih0_sb, stage_rhs_l0(cn_))]
                        xw_tiles0[cn_] = new_xw_tile()
                        st["xw"] = xw_tiles0[cn_]
                    bias_col = 0
                else:
                    def stage():
                        orhs, prhs = stage_rhs_l1(cn_)
                        st["r"] = [(wih1o_sb, orhs), (wih1p_sb, prhs)]
                        xw_tiles1[cn_] = new_xw_tile()
                        st["xw"] = xw_tiles1[cn_]
                    bias_col = GT
                hooks[0] = stage

                parts = []

                def mk(gidx):
                    holder = {}

                    def part_a():
                        a, b = proj_tile_parts(gidx, st["r"], st["xw"],
                                               bias_col)
                        holder["b"] = b
                        a()

                    def part_b():
                        holder["b"]()
                    return part_a, part_b

                for g in range(GT):
                    a, b = mk(g)
                    parts.append(a)
                    parts.append(b)
                # 32 parts over slots 2..TC-1 (30 slots): double-up the
                # first two slots
                slots = list(range(2, TC))
                while len(parts) > len(slots):
                    f1, f2 = parts[0], parts[1]
                    parts = [lambda f1=f1, f2=f2: (f1(), f2())] + parts[2:]
                for sl, fn in zip(slots, parts):
                    hooks[sl] = fn
                return hooks

            # ---------- phase sequence ----------
            # prologue: proj chunk 0 of layer 0
            xw_tiles0 = [None] * NC
            xw_tiles1 = [None] * NC
            m_tiles = [None] * NC
            m_tiles[0] = new_mask_tile(0)
            rhs0 = stage_rhs_l0(0)
            xw_tiles0[0] = new_xw_tile()
            for g in range(GT):
                proj_tile(g, [(wih0_sb, rhs0)], xw_tiles0[0], bias_col=0)

            scan(whh0_sb, xw_tiles0, m_tiles, layer=0)

            # exchange
            nc.gpsimd.collective_compute(
                "AllGather", mybir.AluOpType.bypass,
                ins=[y0x.opt()], outs=[ag.opt()],
                replica_groups=[list(range(NCORES))],
            )

            # prologue: proj chunk 0 of layer 1
            m_tiles2 = [None] * NC
            m_tiles2[0] = new_mask_tile(0)
            orhs0, prhs0 = stage_rhs_l1(0)
            xw_tiles1[0] = new_xw_tile()
            for g in range(GT):
                proj_tile(g, [(wih1o_sb, orhs0), (wih1p_sb, prhs0)],
                          xw_tiles1[0], bias_col=GT)

            scan(whh1_sb, xw_tiles1, m_tiles2, layer=1)

    nc.compile()
    return nc


def _prep_inputs(x, lengths, weights, t_steps=T):
    """Build the 8 per-core input maps."""
    NC = t_steps // TC
    x = np.asarray(x, np.float32)
    lengths = np.asarray(lengths)
    active = (np.arange(T)[:, None] < lengths[None, :]).astype(np.float32)

    per_dir = {}
    for d, pre in ((0, "f"), (1, "b")):
        xs = x if d == 0 else x[::-1]
        am = active if d == 0 else active[::-1]
        xs = xs[:t_steps]
        am = am[:t_steps]
        xTd = np.ascontiguousarray(
            xs.transpose(2, 0, 1)).astype(bf16)  # [D, t, B]
        # mask rows: chunk c covers steps [c*TC, c*TC+TC] inclusive
        amp = np.vstack([am, np.ones((1, B), np.float32)])
        W_ih0 = np.asarray(weights[f"{pre}W_ih0"], np.float32)
        W_hh0 = np.asarray(weights[f"{pre}W_hh0"], np.float32)
        W_ih1 = np.asarray(weights[f"{pre}W_ih1"], np.float32)
        W_hh1 = np.asarray(weights[f"{pre}W_hh1"], np.float32)
        own = W_ih1[:, :H] if d == 0 else W_ih1[:, H:]
        par = W_ih1[:, H:] if d == 0 else W_ih1[:, :H]
        per_dir[d] = dict(
            xT=xTd, amp=amp,
            whh0T=np.ascontiguousarray(
                W_hh0.T.reshape(KH, 128, G)).astype(bf16),
            wih0T=np.ascontiguousarray(
                W_ih0.T.reshape(KH, 128, G)).astype(bf16),
            whh1T=np.ascontiguousarray(
                W_hh1.T.reshape(KH, 128, G)).astype(bf16),
            wih1oT=np.ascontiguousarray(
                own.T.reshape(KH, 128, G)).astype(bf16),
            wih1pT=np.ascontiguousarray(
                par.T.reshape(KH, 128, G)).astype(bf16),
            b0c=np.ascontiguousarray(
                np.asarray(weights[f"{pre}b0"],
                           np.float32).reshape(GT, 128)),
            b1c=np.ascontiguousarray(
                np.asarray(weights[f"{pre}b1"],
                           np.float32).reshape(GT, 128)),
        )

    in_maps = []
    for core in range(NCORES):
        d, s = core // 4, core % 4
        bsl = slice(s * BS, (s + 1) * BS)
        pd = per_dir[d]
        ams = pd["amp"][:, bsl]  # [t_steps+1, BS]
        maskrows = np.empty((NC, (TC + 1) * KH * BS), np.float32)
        for c in range(NC):
            blk = ams[c * TC:c * TC + TC + 1]            # [TC+1, BS]
            maskrows[c] = np.tile(blk, (1, KH)).reshape(-1)
        in_maps.append({
            "ident": np.eye(128, dtype=bf16),
            "xT": np.ascontiguousarray(pd["xT"][:, :, bsl]),
            "maskd": maskrows.astype(bf16),
            "whh0T": pd["whh0T"],
            "wih0T": pd["wih0T"],
            "whh1T": pd["whh1T"],
            "wih1oT": pd["wih1oT"],
            "wih1pT": pd["wih1pT"],
            "b0c": pd["b0c"],
            "b1c": pd["b1c"],
        })
    return in_maps


def _assemble(results, lengths, t_steps=T):
    NC = t_steps // TC
    lengths = np.asarray(lengths)
    active = (np.arange(t_steps)[:, None] < lengths[None, :])
    out = np.zeros((t_steps, B, 2 * H), np.float32)
    for core in range(NCORES):
        d, s = core // 4, core % 4
        arr = np.asarray(results[core]["y1"], dtype=bf16).astype(np.float32)
        # [KH, NC, 128, TC, BS] -> [t, b, j*128+p]
        blk = arr.transpose(1, 3, 4, 0, 2).reshape(t_steps, BS, H)
        if d == 1:
            blk = blk[::-1]
        out[:, s * BS:(s + 1) * BS, d * H:(d + 1) * H] = blk
    out *= active[:, :, None]
    return out


def kernel(x, lengths, fW_ih0, fW_hh0, fb0, bW_ih0, bW_hh0, bb0,
           fW_ih1, fW_hh1, fb1, bW_ih1, bW_hh1, bb1, _t_steps=T,
           _want_trace=False):
    weights = dict(fW_ih0=fW_ih0, fW_hh0=fW_hh0, fb0=fb0,
                   bW_ih0=bW_ih0, bW_hh0=bW_hh0, bb0=bb0,
                   fW_ih1=fW_ih1, fW_hh1=fW_hh1, fb1=fb1,
                   bW_ih1=bW_ih1, bW_hh1=bW_hh1, bb1=bb1)
    key = _t_steps
    if key not in _compiled:
        _compiled[key] = _build(_t_steps)
    nc = _compiled[key]
    in_maps = _prep_inputs(x, lengths, weights, _t_steps)
    res = bass_utils.run_bass_kernel_spmd(
        nc, in_maps, core_ids=list(range(NCORES)), trace=_want_trace)
    out = _assemble(res.results, lengths, _t_steps)
    if _want_trace:
        kernel.last_results = res
    return out
